# revision 29
# baseline (speedup 1.0000x reference)
"""Trainium2 Bass kernel for nn_MultiHeadAttention_38027640439053.

Reference computation (per batch b of 8, one NeuronCore each):
    data = X.reshape(n, 16, 64)
    q/k/v = data @ W{q,k,v}.T          (per-head shared 64x64 weights)
    scores = (q @ k.T per head) / 32
    attn = softmax(scores, axis=k)
    Y = (attn @ v).reshape(n, 1024) @ Wo.T + bo

Kernel strategy (batch-parallel over 8 cores, zero collectives):
  - X is converted to bf16 on DVE, then transposed on-chip via PE
    transposes (XT: emb on partitions).
  - Q/K projected two-heads-at-a-time with block-diag(W.T) stationary
    weights -> QT/KT in [head_dim, n] layout.
  - Scores computed TRANSPOSED: ST[k, q] = K Q^T per head, two heads
    run concurrently in the PE array via row-tiling (K=64 each).
  - exp() on ScalarE directly from PSUM with the 1/32 scale folded in.
    No max-subtraction (scores have sigma ~0.25; exp range ~[0.2, 5]).
  - P@V computed as YT = V^T P^T with V in row layout augmented by a
    ones column -> row 64 of the output is the softmax denominator D.
  - Normalization deferred: recip(D) on VectorE, broadcast across
    partitions with a 0/1 selector matmul, one multiply per slab.
  - Software pipelining: the ST+exp phase of pair p is ACT-bound
    (ScalarE exp is slower than the PE making scores), so pair
    p-1's P@V matmuls and pair p+1's projections are interleaved into
    pair p's score loop to keep the PE busy.
  - Output projection accumulates over 8 emb chunks per n-tile; the
    bias is added with a rank-1 (ones x bo) matmul into the same PSUM
    accumulation group. Wo^T streams into recycled PT slots at the
    tail.

bf16 is used for all the large matmuls: full PE rate at every free
dim (fp32r drops to 1/4 rate below 256-wide), 2x faster DVE copies,
2x faster weight loads (FWL needs a non-fp32 dtype), and half the
SBUF/DMA traffic. rel-err budget is 2e-2; bf16 lands ~1e-3..1e-2.
"""

import numpy as np
import ml_dtypes

import concourse.bacc as bacc
import concourse.mybir as mybir
import concourse.tile as tile
from concourse.bass_utils import run_bass_kernel_spmd

F32 = mybir.dt.float32
BF16 = mybir.dt.bfloat16

EXP = mybir.ActivationFunctionType.Exp

# stages quantized to bf16
DEFAULT_FAST = frozenset({"proj", "st", "pv", "outp", "bias", "bcast"})


class _PoolPfx:
    """Wrap a tile pool, prefixing tile names so multiple reps can
    share one pool (tags & ring buffers deliberately shared: that is
    what lets consecutive repeat-bodies pipeline without a pool-close
    barrier)."""

    def __init__(self, pool, pfx):
        self._pool, self._pfx = pool, pfx

    def tile(self, shape, dtype, name, tag, bufs=None):
        return self._pool.tile(shape, dtype, name=self._pfx + name, tag=tag,
                               bufs=bufs)


def emit_body(tc, nc, aps, N, EMB, NH, fast, rep, pools, skip=frozenset(),
              with_bias=True):
    NPAIR = NH // 2
    NT = N // 128        # n tiles (rows of X / q tiles)
    KT = N // 128        # k tiles
    assert EMB == NPAIR * 128
    scale = 1.0 / float(np.sqrt(EMB))
    qch = [(s, min(512, N - s)) for s in range(0, N, 512)]
    ech = [(s, min(512, EMB - s)) for s in range(0, EMB, 512)]
    KT_PER = min(4, KT)
    assert KT % KT_PER == 0
    NPT = KT // KT_PER   # PT tiles per head

    X_d, WqT2_d, WkT2_d, WvT2_d, WoT_d, bo_d, sel_d, ones_d, iden_d, Y_d = aps

    dt_w = BF16 if "proj" in fast else F32    # wq2/wk2/wv2 + xsb/xt/iden
    dt_qk = BF16 if "st" in fast else F32     # qt/kt
    dt_pv = BF16 if "pv" in fast else F32     # pt/vslab
    dt_o = BF16 if "outp" in fast else F32    # yt/wot
    dt_b = BF16 if "bias" in fast else F32    # ones/bo
    dt_r = BF16 if "bcast" in fast else F32   # sel/rd/dh

    pfx = f"r{rep}_"
    consts = _PoolPfx(pools["consts"], pfx)
    xp = _PoolPfx(pools["xp"], pfx)
    xtp = _PoolPfx(pools["xtp"], pfx)
    qkp = _PoolPfx(pools["qkp"], pfx)
    vp = _PoolPfx(pools["vp"], pfx)
    ptp = _PoolPfx(pools["ptp"], pfx)
    ytp = _PoolPfx(pools["ytp"], pfx)
    rdp = _PoolPfx(pools["rdp"], pfx)
    stps = _PoolPfx(pools["stps"], pfx)
    mps = _PoolPfx(pools["mps"], pfx)
    if True:
        # ---- X column slabs, loaded just in time per pair ----
        # slab p holds X[:, p*128:(p+1)*128] as [128 rows-of-ntile, NT*128]:
        # [part, i*128 + c] = X[i*128 + part, p*128 + c]
        x_slabs = {}

        def load_x(p, split=1):
            xs = xp.tile([128, NT * 128], F32, name=f"x{p}", tag="x")
            h = (NT * 128) // split
            for j in range(split):
                nc.sync.dma_start(
                    out=xs[:, j * h:(j + 1) * h]
                    .rearrange("p (i c) -> p i c", c=128),
                    in_=X_d[j * (N // split):(j + 1) * (N // split),
                            p * 128:(p + 1) * 128]
                    .rearrange("(i p) c -> p i c", p=128))
            x_slabs[p] = xs

        load_x(0, split=4)

        # ---- constants: identical for every rep, loaded once ----
        if "_consts" not in pools:
            iden = consts.tile([128, 128], dt_w, name="iden", tag="iden")
            nc.sync.dma_start(out=iden[:], in_=iden_d[:])
            wq2 = consts.tile([128, 128], dt_w, name="wq2", tag="wq2")
            nc.sync.dma_start(out=wq2[:], in_=WqT2_d[:])
            wk2 = consts.tile([128, 128], dt_w, name="wk2", tag="wk2")
            nc.sync.dma_start(out=wk2[:], in_=WkT2_d[:])
            wv2 = consts.tile([128, 128], dt_w, name="wv2", tag="wv2")
            nc.sync.dma_start(out=wv2[:], in_=WvT2_d[:])
            bo_t = consts.tile([1, EMB], dt_b, name="bo_t", tag="bo_t")
            nc.sync.dma_start(out=bo_t[:], in_=bo_d[:])
            ones_t = consts.tile([1, 128], dt_b, name="ones_t", tag="ones_t")
            nc.sync.dma_start(out=ones_t[:], in_=ones_d[:])
            sel_t = xp.tile([NH, EMB], dt_r, name="sel_t", tag="selx")
            nc.sync.dma_start(out=sel_t[:], in_=sel_d[:])
            pools["_consts"] = (iden, wq2, wk2, wv2, bo_t, ones_t, sel_t)
        iden, wq2, wk2, wv2, bo_t, ones_t, sel_t = pools["_consts"]

        # raw softmax denominators, one row per head; reciprocal'd once
        # at the tail (a per-head [1, N] reciprocal runs on a single DVE
        # lane at ~12 cyc/elem = 6.5us each -- batching all 16 rows into
        # one op + reciprocal_approx_fast makes it ~1.5us total).
        rd = rdp.tile([NH, N], F32, name="rd", tag="rd")

        def touch(dst, src):
            # minimal write so the tile allocator materializes `dst`
            nc.scalar.activation(dst, src,
                                 mybir.ActivationFunctionType.Copy,
                                 bias=0.0, scale=0.0)

        def build_xt(p, psp=None):
            """XT slab p: [128 emb dims of pair p, N] via 8 PE transposes.

            Pair 0 stages its PSUM in the score pool (psp=stps): at a
            repeat boundary the mps ring is still held by the previous
            body's outproj accumulators, which otherwise stalls the next
            body's whole prologue chain ~8us."""
            xs = x_slabs.pop(p)
            xsb = xtp.tile([128, N], dt_w, name=f"xsb{p}", tag="xt")
            for j in range(4):
                nc.vector.tensor_copy(xsb[:, j * (N // 4):(j + 1) * (N // 4)],
                                      xs[:, j * (N // 4):(j + 1) * (N // 4)])
            xt_ps = (psp or mps).tile([128, N], dt_w, name=f"xtps{p}",
                                      tag="st" if psp else "m")
            if "proj" not in skip:
                for i in range(NT):
                    nc.tensor.transpose(
                        xt_ps[:, i * 128:(i + 1) * 128],
                        xsb[:, i * 128:(i + 1) * 128],
                        iden[:],
                    )
            else:
                # bf16 PSUM can only be written by matmul/transpose
                nc.tensor.transpose(xt_ps[:, 0:128], xsb[:, 0:128], iden[:])
            xt = xtp.tile([128, N], dt_w, name=f"xt{p}", tag="xt")
            nc.vector.tensor_copy(xt[:], xt_ps[:])
            return xt

        def proj_qk(p, xt, psp=None):
            pool, tg = (psp, "st") if psp else (mps, "m")
            qps = pool.tile([128, N], F32, name=f"qps{p}", tag=tg)
            if "proj" not in skip:
                for (s, w) in qch:
                    nc.tensor.matmul(qps[:, s:s + w], wq2[:], xt[:, s:s + w])
            else:
                touch(qps[:, 0:16], iden[:, 0:16])
            qt = qkp.tile([128, N], dt_qk, name=f"qt{p}", tag="qk")
            nc.vector.tensor_copy(qt[:], qps[:])

            kps = pool.tile([128, N], F32, name=f"kps{p}", tag=tg)
            if "proj" not in skip:
                for (s, w) in qch:
                    nc.tensor.matmul(kps[:, s:s + w], wk2[:], xt[:, s:s + w])
            else:
                touch(kps[:, 0:16], iden[:, 0:16])
            kt = qkp.tile([128, N], dt_qk, name=f"kt{p}", tag="qk")
            nc.vector.tensor_copy(kt[:], kps[:])
            return qt, kt

        def proj_v(p, xt, psp=None):
            # V in row layout: [n, 2 heads x 64]
            pool, tg = (psp, "st") if psp else (mps, "m")
            vps = pool.tile([128, N], F32, name=f"vps{p}", tag=tg)
            if "proj" not in skip:
                for i in range(NT):
                    nc.tensor.matmul(vps[:, i * 128:(i + 1) * 128],
                                     xt[:, i * 128:(i + 1) * 128], wv2[:])
            else:
                touch(vps[:, 0:16], iden[:, 0:16])
            vslab = vp.tile([128, KT * 130], dt_pv, name=f"vslab{p}", tag="v")
            vv = vslab[:].rearrange("p (j c) -> p j c", c=130)
            vs = vps[:].rearrange("p (j c) -> p j c", c=128)
            nc.vector.tensor_copy(vv[:, :, 0:64], vs[:, :, 0:64])
            nc.vector.tensor_copy(vv[:, :, 65:129], vs[:, :, 64:128])
            v4 = vslab[:].rearrange("p (j k c) -> p j k c", k=2, c=65)
            ones_src = iden[:, 0:2 * KT].rearrange("p (j k c) -> p j k c",
                                                   k=2, c=1)
            nc.scalar.activation(v4[:, :, :, 64:65], ones_src,
                                 mybir.ActivationFunctionType.Copy,
                                 bias=1.0, scale=0.0)
            return vslab

        def st_exp(p, ktile, qt, kt, pts):
            """Scores (transposed) + exp for one k-tile, both heads.

            The two heads use disjoint PE row groups (rows 0-63 / 64-127
            via base_partition-derived tile_position) and different PSUM
            banks; chunk matmuls are interleaved head0/head1 so the
            hardware can overlap them."""
            sts = {}
            for head in (0, 1):
                sts[head] = stps.tile([128, N], F32,
                                      name=f"st{p}_{ktile}_{head}",
                                      tag="st")
            if "st" not in skip:
                for (s, w) in qch:
                    for head in (0, 1):
                        r0 = head * 64
                        nc.tensor.matmul(
                            sts[head][:, s:s + w],
                            kt[r0:r0 + 64, ktile * 128:(ktile + 1) * 128],
                            qt[r0:r0 + 64, s:s + w],
                        )
            else:
                for head in (0, 1):
                    touch(sts[head][:, 0:16], iden[:, 0:16])
            for head in (0, 1):
                st = sts[head]
                if ktile % KT_PER == 0:
                    pt = ptp.tile([128, KT_PER * N], dt_pv,
                                  name=f"pt{p}_{head}_{ktile // KT_PER}",
                                  tag="pt")
                    pts[head].append(pt)
                dst = pts[head][-1][:, (ktile % KT_PER) * N:
                                    (ktile % KT_PER + 1) * N]
                if "exp" not in skip:
                    nc.scalar.activation(dst, st[:], EXP, scale=scale)
                else:
                    touch(dst[:, 0:16], st[:, 0:16])

        # pipeline state for the deferred PV of the previous pair
        pv_state = {}

        def pv_quarter(p, head, half, vslab, pts):
            """8 accumulating matmuls: chunks [half*KT/2, (half+1)*KT/2)."""
            if half == 0:
                pv_state[(p, head)] = mps.tile(
                    [65, N], F32, name=f"pvps{p}_{head}", tag="m")
            pvps = pv_state[(p, head)]
            k0, k1 = half * (KT // 2), (half + 1) * (KT // 2)
            if "pv" in skip:
                if half == 0:
                    touch(pvps[:, 0:16], iden[0:65, 0:16])
                return
            for ktile in range(k0, k1):
                pt = pts[head][ktile // KT_PER]
                base = (ktile % KT_PER) * N
                lhs = vslab[:, ktile * 130 + head * 65:
                            ktile * 130 + head * 65 + 65]
                for (s, w) in qch:
                    nc.tensor.matmul(
                        pvps[:, s:s + w], lhs,
                        pt[:, base + s:base + s + w],
                        start=(ktile == 0), stop=(ktile == KT - 1),
                    )

        def finish_head(p, head, yt):
            pvps = pv_state.pop((p, head))
            nc.vector.tensor_copy(yt[head * 64:head * 64 + 64, :],
                                  pvps[0:64, :])
            # stage raw D at its native partition (DVE is lane-aligned),
            # then DMA the row into the [NH, N] collection tile.
            dh = xtp.tile([65, N], F32, name=f"dh{p}_{head}", tag="xt")
            nc.vector.tensor_copy(dh[64:65, :], pvps[64:65, :])
            nc.sync.dma_start(out=rd[2 * p + head:2 * p + head + 1, :],
                              in_=dh[64:65, :])

        def bcast_mul(p, yt, rdb):
            # stps banks are idle once the last exp has drained
            bps = stps.tile([128, N], F32, name=f"bps{p}", tag="st")
            for (s, w) in qch:
                nc.tensor.matmul(bps[:, s:s + w],
                                 sel_t[:, p * 128:(p + 1) * 128],
                                 rdb[:, s:s + w])
            nc.vector.tensor_mul(yt[:], yt[:], bps[:])

        rdb = rdp.tile([NH, N], dt_r, name="rdb", tag="rdb")
        # the early bcast matmuls contract over all NH rows of rdb while
        # the last pair's rows are not yet written: zero once so garbage
        # NaNs can't leak through the selector's 0-entries.
        nc.scalar.activation(rdb[:, :], sel_t[:, 0:N],
                             mybir.ActivationFunctionType.Copy,
                             bias=0.0, scale=0.0)

        # ---------------- pipelined pair loop ----------------
        yts = []
        all_pts = {}
        vslabs = {}

        # pair 0 prologue: xt0 built once, q/k/v projections
        xt0 = build_xt(0)
        cur_qt, cur_kt = proj_qk(0, xt0)
        vslabs[0] = proj_v(0, xt0)
        nxt = {}
        for p in range(NPAIR):
            pts = {0: [], 1: []}
            all_pts[p] = pts
            yts.append(ytp.tile([128, N], dt_o, name=f"yt{p}", tag="yt"))

            sched = {k: [] for k in range(KT)}
            if p > 0:
                po, vo, pp = p - 1, vslabs[p - 1], all_pts[p - 1]
                yo = yts[p - 1]
                tasks = [
                    lambda: pv_quarter(po, 0, 0, vo, pp),
                    lambda: (pv_quarter(po, 0, 1, vo, pp),
                             finish_head(po, 0, yo)),
                    lambda: pv_quarter(po, 1, 0, vo, pp),
                    lambda: (pv_quarter(po, 1, 1, vo, pp),
                             finish_head(po, 1, yo)),
                ]
                for j, pos in enumerate((0, KT // 4, KT // 2,
                                         (3 * KT) // 4)):
                    sched[min(KT - 1, pos)].append(tasks[j])

            if p + 1 < NPAIR:
                pn = p + 1
                tasks = [
                    lambda: load_x(pn),
                    lambda: nxt.__setitem__("xt", build_xt(pn)),
                    lambda: nxt.__setitem__("qk", proj_qk(pn, nxt["xt"])),
                    lambda: vslabs.__setitem__(pn, proj_v(pn, nxt["xt"])),
                ]
                for j, pos in enumerate((0, 1, 3, 5)):
                    sched[max(0, pos)].append(tasks[j])
            for ktile in range(KT):
                for t in sched[ktile]:
                    t()
                st_exp(p, ktile, cur_qt, cur_kt, pts)
            if p - 1 >= 0:
                del vslabs[p - 1], all_pts[p - 1]
            if p + 1 < NPAIR:
                cur_qt, cur_kt = nxt["qk"]

        # ---------------- tail: last pair's PV + Wo load + outproj ----
        last = NPAIR - 1
        pv_quarter(last, 0, 0, vslabs[last], all_pts[last])
        pv_quarter(last, 0, 1, vslabs[last], all_pts[last])
        finish_head(last, 0, yts[last])
        # WoT streams into recycled PT slots ([128, KT_PER*N] tiles)
        cpt = (KT_PER * N) // EMB      # Wo chunks per PT-sized tile
        n_wt = (NPAIR + cpt - 1) // cpt
        wot_tiles = []
        for t in range(n_wt):
            wt = ptp.tile([128, KT_PER * N], dt_o, name=f"wotT{t}", tag="pt")
            nct = min(cpt, NPAIR - t * cpt)
            nc.sync.dma_start(
                out=wt[:, 0:nct * EMB].rearrange("p (c e) -> p c e", e=EMB),
                in_=WoT_d[t * cpt * 128:(t * cpt + nct) * 128, :]
                .rearrange("(c p) e -> p c e", p=128))
            wot_tiles.append(wt)
        # normalize pairs 0..NPAIR-2 (their denominators are final) in
        # parallel with the last pair's head-1 PV below.
        rdf = rdp.tile([NH, N], F32, name="rdf", tag="rdf")
        nc.vector.reciprocal_approx_fast(rdf[0:2 * last, :],
                                         rd[0:2 * last, :])
        nc.vector.tensor_copy(rdb[0:2 * last, :], rdf[0:2 * last, :])
        for p in range(last):
            bcast_mul(p, yts[p], rdb)
        pv_quarter(last, 1, 0, vslabs[last], all_pts[last])
        pv_quarter(last, 1, 1, vslabs[last], all_pts[last])
        finish_head(last, 1, yts[last])
        rdf2 = rdp.tile([NH, N], F32, name="rdf2", tag="rdf")
        nc.vector.reciprocal_approx_fast(rdf2[:], rd[:])
        rdb2 = rdp.tile([NH, N], dt_r, name="rdb2", tag="rdb2")
        nc.vector.tensor_copy(rdb2[:], rdf2[:])
        bcast_mul(last, yts[last], rdb2)

        def wot_ap(p):
            return wot_tiles[p // cpt][:, (p % cpt) * EMB:
                                       (p % cpt + 1) * EMB]

        for i in range(NT):
            ops = mps.tile([128, EMB], F32, name=f"ops{i}", tag="m")
            if "outp" in skip:
                touch(ops[:, 0:16], iden[:, 0:16])
            if "outp" not in skip:
                for p in range(NPAIR):
                    wchunk = wot_ap(p)
                    for (s, w) in ech:
                        nc.tensor.matmul(
                            ops[:, s:s + w],
                            yts[p][:, i * 128:(i + 1) * 128],
                            wchunk[:, s:s + w],
                            start=(p == 0),
                            stop=(not with_bias and p == NPAIR - 1),
                        )
                if with_bias:
                    for (s, w) in ech:
                        nc.tensor.matmul(ops[:, s:s + w], ones_t[:],
                                         bo_t[:, s:s + w],
                                         start=False, stop=True)
            osb = xp.tile([128, EMB], F32, name=f"osb{i}", tag="x")
            # alternate the PSUM->SBUF drain between DVE and the (tail-
            # idle) ScalarE so the eight copies don't serialize on DVE
            if i % 2 == 0:
                nc.vector.tensor_copy(osb[:], ops[:])
            else:
                nc.scalar.activation(osb[:], ops[:],
                                     mybir.ActivationFunctionType.Copy,
                                     bias=0.0, scale=1.0)
            nc.sync.dma_start(out=Y_d[i * 128:(i + 1) * 128, :], in_=osb[:])


def build_program(N=1024, EMB=1024, NH=16, n_cores=8, fast=DEFAULT_FAST,
                  repeat=1, trace_sim=False, skip=frozenset(),
                  with_bias=True):
    dt_w = BF16 if "proj" in fast else F32
    dt_o = BF16 if "outp" in fast else F32
    dt_b = BF16 if "bias" in fast else F32
    dt_r = BF16 if "bcast" in fast else F32
    nc = bacc.Bacc("TRN2", target_bir_lowering=False, debug=False,
                   num_devices=n_cores)
    aps = (
        nc.dram_tensor("X", [N, EMB], F32, kind="ExternalInput").ap(),
        nc.dram_tensor("WqT2", [128, 128], dt_w, kind="ExternalInput").ap(),
        nc.dram_tensor("WkT2", [128, 128], dt_w, kind="ExternalInput").ap(),
        nc.dram_tensor("WvT2", [128, 128], dt_w, kind="ExternalInput").ap(),
        nc.dram_tensor("WoT", [EMB, EMB], dt_o, kind="ExternalInput").ap(),
        nc.dram_tensor("bo", [1, EMB], dt_b, kind="ExternalInput").ap(),
        nc.dram_tensor("sel", [NH, EMB], dt_r, kind="ExternalInput").ap(),
        nc.dram_tensor("ones", [1, 128], dt_b, kind="ExternalInput").ap(),
        nc.dram_tensor("iden", [128, 128], dt_w, kind="ExternalInput").ap(),
        nc.dram_tensor("Y", [N, EMB], F32, kind="ExternalOutput").ap(),
    )
    NPAIR = NH // 2
    with tile.TileContext(nc, trace_sim=trace_sim) as tc:
        with (
            tc.tile_pool(name="consts", bufs=1) as consts,
            tc.tile_pool(name="xp", bufs=4) as xp,
            tc.tile_pool(name="xtp", bufs=4) as xtp,
            tc.tile_pool(name="qkp", bufs=4) as qkp,
            tc.tile_pool(name="vp", bufs=3) as vp,
            tc.tile_pool(name="ptp", bufs=5) as ptp,
            tc.tile_pool(name="ytp", bufs=NPAIR) as ytp,
            tc.tile_pool(name="rdp", bufs=2) as rdp,
            tc.tile_pool(name="stps", bufs=2, space="PSUM") as stps,
            tc.tile_pool(name="mps", bufs=2, space="PSUM") as mps,
        ):
            pools = dict(consts=consts, xp=xp, xtp=xtp, qkp=qkp, vp=vp,
                         ptp=ptp, ytp=ytp, rdp=rdp, stps=stps, mps=mps)
            for rep in range(repeat):
                emit_body(tc, nc, aps, N, EMB, NH, fast, rep, pools,
                          skip=skip, with_bias=with_bias)
    nc.compile()
    return nc


def host_consts(Wq, Wk, Wv, Wo, bo, NH=16, fast=DEFAULT_FAST):
    EMB = NH * 64
    bf = ml_dtypes.bfloat16

    def cast(a, stage):
        return a.astype(bf) if stage in fast else a

    def blk2(W):
        out = np.zeros((128, 128), np.float32)
        out[0:64, 0:64] = W.T
        out[64:128, 64:128] = W.T
        return out

    sel = np.zeros((NH, EMB), np.float32)
    for p in range(NH // 2):
        sel[2 * p, p * 128:p * 128 + 64] = 1.0
        sel[2 * p + 1, p * 128 + 64:p * 128 + 128] = 1.0
    return {
        "WqT2": cast(blk2(np.asarray(Wq, np.float32)), "proj"),
        "WkT2": cast(blk2(np.asarray(Wk, np.float32)), "proj"),
        "WvT2": cast(blk2(np.asarray(Wv, np.float32)), "proj"),
        "WoT": cast(np.ascontiguousarray(np.asarray(Wo, np.float32).T),
                    "outp"),
        "bo": cast(np.asarray(bo, np.float32).reshape(1, EMB), "bias"),
        "sel": cast(sel, "bcast"),
        "ones": cast(np.ones((1, 128), np.float32), "bias"),
        "iden": cast(np.eye(128, dtype=np.float32), "proj"),
    }


_NC_CACHE = {}


def kernel(X, Wq, Wk, Wv, Wo, bo):
    X = np.asarray(X, np.float32)
    B, N, EMB = X.shape
    NH = EMB // 64
    with_bias = bool(np.any(np.asarray(bo, np.float32)))
    key = (N, EMB, NH, B, with_bias)
    if key not in _NC_CACHE:
        _NC_CACHE[key] = build_program(N=N, EMB=EMB, NH=NH, n_cores=B,
                                       with_bias=with_bias)
    nc = _NC_CACHE[key]
    consts = host_consts(Wq, Wk, Wv, Wo, bo, NH=NH)
    in_maps = [dict(consts, X=np.ascontiguousarray(X[c])) for c in range(B)]
    res = run_bass_kernel_spmd(nc, in_maps, list(range(B)))
    return np.stack([res.results[c]["Y"] for c in range(B)], axis=0)


if __name__ == "__main__":
    rng = np.random.default_rng(0)
    B, N, EMB, NH = 8, 1024, 1024, 16
    X = rng.standard_normal((B, N, EMB), dtype=np.float32)
    Wq = (rng.standard_normal((64, 64), dtype=np.float32) / 8)
    Wk = (rng.standard_normal((64, 64), dtype=np.float32) / 8)
    Wv = (rng.standard_normal((64, 64), dtype=np.float32) / 8)
    Wo = (rng.standard_normal((EMB, EMB), dtype=np.float32) / 32)
    bo = np.zeros(EMB, np.float32)
    Y = kernel(X=X, Wq=Wq, Wk=Wk, Wv=Wv, Wo=Wo, bo=bo)
    print("OK", Y.shape, Y.dtype)
